# revision 3
# baseline (speedup 1.0000x reference)
"""DGCNN (4x EdgeConv + final 1x1 conv) Bass/Tile kernel for Trainium2.

Sharding: pure data parallel — one point cloud (batch element) per
NeuronCore, conv weights replicated. B=8, N=2048, K=20.

On-chip layout is channels-first: activations live as [C, N] SBUF tiles so
that

  * kNN distance tiles come straight out of the tensor engine
    (dist = X_tile^T @ X  -> [128 points, 2048 candidates] in PSUM),
  * per-row exact top-20 runs on the vector engine via max8 / max_index /
    match_replace8 rounds,
  * neighbor feature gathering is a free-dim gather shared across channels
    (gpsimd ap_gather),
  * EdgeConv is two accumulating matmuls (neighbor part + center part, the
    "x_j - x_i" concat algebra is folded into the weights on the host),
  * max over k=20 neighbors is a strided reduce straight out of PSUM.

Exactness note: top-20 via 3 rounds of (max8, max_index, match_replace8)
matches jax.lax.top_k up to f32 ties, and max-over-k is order invariant.
LeakyReLU is monotone, so it is applied after the k-max; the BN eval scale g
is folded into the conv weights on the host and the bias is added after the
k-max reduce (also commutes with max).
"""

import sys

import numpy as np

for _p in ("/opt/trn_rl_repo", "/root/.axon_site/_ro/trn_rl_repo"):
    if _p not in sys.path:
        sys.path.insert(0, _p)

N = 2048
KNN = 20
NT = 16  # row tiles of 128 points
NCORES = 8

# (Cin_padded, Cout) per EdgeConv layer; Cin is the channels-first row count
# of the input activation buffer (layer 1 input is 3 coords padded to 16).
LAYERS = [
    dict(cin=16, cout=64),
    dict(cin=64, cout=64),
    dict(cin=64, cout=128),
    dict(cin=128, cout=256),
]

_CACHE = {}
LAST_RESULTS = None  # BassKernelResults of the most recent run (for profiling)


def _build_program():
    import concourse.bass as bass  # noqa: F401
    import concourse.mybir as mybir
    import concourse.tile as tile
    from concourse import bacc
    from contextlib import ExitStack

    f32 = mybir.dt.float32
    u16 = mybir.dt.uint16
    i16 = mybir.dt.int16

    nc = bacc.Bacc("TRN2", target_bir_lowering=False, debug=False,
                   enable_asserts=False)

    # ---- DRAM I/O ----
    x0_d = nc.dram_tensor("x0", [16, N], f32, kind="ExternalInput").ap()
    wd_d, wc_d, b_d = [], [], []
    for li, L in enumerate(LAYERS, start=1):
        wd_d.append(nc.dram_tensor(f"wd{li}", [L["cin"], L["cout"]], f32,
                                   kind="ExternalInput").ap())
        wc_d.append(nc.dram_tensor(f"wc{li}", [L["cin"], L["cout"]], f32,
                                   kind="ExternalInput").ap())
        bshape = [min(L["cout"], 128), L["cout"] // 128 if L["cout"] > 128 else 1]
        b_d.append(nc.dram_tensor(f"b{li}", bshape, f32,
                                  kind="ExternalInput").ap())
    w5_d = [nc.dram_tensor(f"w5{t}", [k, 1024], f32, kind="ExternalInput").ap()
            for t, k in (("a", 64), ("b", 64), ("c", 128), ("d", 128), ("e", 128))]
    b5_d = nc.dram_tensor("b5", [128, 8], f32, kind="ExternalInput").ap()
    out_d = nc.dram_tensor("out", [1024, N], f32, kind="ExternalOutput").ap()

    with tile.TileContext(nc) as tc, ExitStack() as ctx:
        wp = ctx.enter_context(tc.tile_pool(name="wp", bufs=1))
        xp = ctx.enter_context(tc.tile_pool(name="xp", bufs=1))
        sb = ctx.enter_context(tc.tile_pool(name="sb", bufs=2))
        psD = ctx.enter_context(tc.tile_pool(name="psD", bufs=2, space="PSUM"))
        psE = ctx.enter_context(tc.tile_pool(name="psE", bufs=1, space="PSUM"))
        dr = ctx.enter_context(tc.tile_pool(name="dr", bufs=2, space="DRAM"))

        def load(dram_ap, tag):
            t = wp.tile(list(dram_ap.shape), dram_ap.dtype, tag=tag)
            nc.sync.dma_start(out=t[:], in_=dram_ap)
            return t

        wd = [load(a, f"wd{i}") for i, a in enumerate(wd_d)]
        wc = [load(a, f"wc{i}") for i, a in enumerate(wc_d)]
        bb = [load(a, f"b{i}") for i, a in enumerate(b_d)]
        w5 = [load(a, f"w5{i}") for i, a in enumerate(w5_d)]
        b5 = load(b5_d, "b5")

        # persistent activation buffers (channels-first)
        X0 = xp.tile([16, N], f32, tag="X0")
        nc.sync.dma_start(out=X0[:], in_=x0_d)
        X1 = xp.tile([64, N], f32, tag="X1")
        X2 = xp.tile([64, N], f32, tag="X2")
        X3 = xp.tile([128, N], f32, tag="X3")
        X4a = xp.tile([128, N], f32, tag="X4a")
        X4b = xp.tile([128, N], f32, tag="X4b")

        xxrow = xp.tile([1, N], f32, tag="xxrow")
        scratch = xp.tile([128, N], f32, tag="scratch")

        ones_c = wp.tile([128, 1], f32, tag="ones_c")
        nc.vector.memset(ones_c[:], 1.0)
        neghalf = wp.tile([1, 128], f32, tag="neghalf")
        nc.vector.memset(neghalf[:], -0.5)

        xin = [X0, X1, X2, X3]
        xout = [[(X1, 0)], [(X2, 0)], [(X3, 0)], [(X4a, 0), (X4b, 0)]]

        for li, L in enumerate(LAYERS):
            cin, cout = L["cin"], L["cout"]
            Xi = xin[li]

            # ---- xx[j] = sum_c Xi[c, j]^2 ----
            nc.scalar.square(out=scratch[0:cin, :], in_=Xi[0:cin, :])
            for q in range(4):
                pxx = psD.tile([128, 512], f32, tag="psD")
                nc.tensor.matmul(pxx[0:1, :], lhsT=ones_c[0:cin, 0:1],
                                 rhs=scratch[0:cin, q * 512:(q + 1) * 512],
                                 start=True, stop=True)
                nc.scalar.copy(out=xxrow[:, q * 512:(q + 1) * 512],
                               in_=pxx[0:1, :])

            for r in range(NT):
                rc = r * 128
                # ---- ranking keys: inner(i,j) - xx[j]/2  (order == neg dist) ----
                dist = sb.tile([128, N], f32, tag="dist")
                for q in range(4):
                    pd = psD.tile([128, 512], f32, tag="psD")
                    cs = q * 512
                    nc.tensor.matmul(pd[:], lhsT=Xi[0:cin, rc:rc + 128],
                                     rhs=Xi[0:cin, cs:cs + 512],
                                     start=True, stop=False)
                    nc.tensor.matmul(pd[:], lhsT=neghalf[:, 0:128],
                                     rhs=xxrow[:, cs:cs + 512],
                                     start=False, stop=True)
                    nc.scalar.copy(out=dist[:, cs:cs + 512], in_=pd[:])

                # ---- exact top-24 (>= top-20) per row ----
                idx = sb.tile([128, 24], u16, tag="idx")
                m8 = sb.tile([128, 8], f32, tag="m8")
                for rnd in range(3):
                    nc.vector.max(out=m8[:], in_=dist[:])
                    nc.vector.max_index(out=idx[:, rnd * 8:(rnd + 1) * 8],
                                        in_max=m8[:], in_values=dist[:])
                    if rnd < 2:
                        nc.vector.match_replace(out=dist[:], in_to_replace=m8[:],
                                                in_values=dist[:],
                                                imm_value=-1e30)

                # ---- shuffle idx[i, k] into ap_gather's wrapped layout ----
                # gather column j = 128*k + i; DRAM scratch holds flat[j],
                # so the wrapped view is (partition j%16, slot j//16).
                scr = dr.tile([KNN, 128], u16, tag="scr")
                nc.sync.dma_start(out=scr[:].rearrange("k i -> i k"),
                                  in_=idx[:, 0:KNN])
                gidx = sb.tile([cin, 160], i16, tag="gidx")
                src = scr[:].rearrange("k (u p) -> p (k u)", p=16).bitcast(i16)
                for g in range(cin // 16):
                    nc.sync.dma_start(out=gidx[g * 16:(g + 1) * 16, :],
                                      in_=src)

                # ---- gather neighbor features: nbr[:, 128k+i] = Xi[:, idx[i,k]] ----
                nbr = sb.tile([cin, KNN * 128], f32, tag="nbr")
                nc.gpsimd.ap_gather(out_ap=nbr[:], in_ap=Xi[0:cin, :],
                                    idxs_ap=gidx[:], channels=cin,
                                    num_elems=N, d=1, num_idxs=KNN * 128)

                # ---- EdgeConv matmuls + k-max ----
                xb = Xi[0:cin, rc:rc + 128].unsqueeze(1).to_broadcast(
                    [cin, 4, 128])
                for m, (Xo, row0) in enumerate(xout[li]):
                    mm = min(128, cout - m * 128)
                    pE = psE.tile([128, KNN * 128], f32, tag="psE")
                    for q in range(5):
                        cs = q * 512
                        nc.tensor.matmul(
                            pE[0:mm, cs:cs + 512],
                            lhsT=wd[li][0:cin, m * 128:m * 128 + mm],
                            rhs=nbr[0:cin, cs:cs + 512],
                            start=True, stop=False)
                        nc.tensor.matmul(
                            pE[0:mm, cs:cs + 512],
                            lhsT=wc[li][0:cin, m * 128:m * 128 + mm],
                            rhs=xb, start=False, stop=True)
                    # max over k: columns 128k+i, k innermost via stride trick
                    nc.vector.tensor_reduce(
                        out=Xo[row0:row0 + mm, rc:rc + 128],
                        in_=pE[0:mm, :].rearrange("c (k i) -> c i k", i=128),
                        axis=mybir.AxisListType.X, op=mybir.AluOpType.max)

            # ---- bias + leaky relu (commute with k-max) ----
            for m, (Xo, row0) in enumerate(xout[li]):
                mm = min(128, cout - m * 128)
                rows = Xo[row0:row0 + mm, :]
                nc.vector.tensor_scalar_add(rows, rows, bb[li][0:mm, m:m + 1])
                nc.vector.tensor_scalar_mul(scratch[0:mm, :], rows, 0.2)
                nc.vector.tensor_tensor(out=rows, in0=rows,
                                        in1=scratch[0:mm, :],
                                        op=mybir.AluOpType.max)

        # ---- final 1x1 conv: out = leaky(cat @ W5g + b5), channels-first ----
        ktiles = [(X1, 64, w5[0]), (X2, 64, w5[1]), (X3, 128, w5[2]),
                  (X4a, 128, w5[3]), (X4b, 128, w5[4])]
        for m in range(8):
            hst = sb.tile([128, N], f32, tag="dist")
            for q in range(4):
                cs = q * 512
                pF = psD.tile([128, 512], f32, tag="psD")
                for t, (Xt, kdim, Wt) in enumerate(ktiles):
                    nc.tensor.matmul(pF[:], lhsT=Wt[0:kdim, m * 128:(m + 1) * 128],
                                     rhs=Xt[0:kdim, cs:cs + 512],
                                     start=(t == 0), stop=(t == len(ktiles) - 1))
                nc.scalar.activation(out=hst[:, cs:cs + 512], in_=pF[:],
                                     func=mybir.ActivationFunctionType.Identity,
                                     bias=b5[:, m:m + 1], scale=1.0)
            nc.vector.tensor_scalar_mul(scratch[:], hst[:], 0.2)
            nc.vector.tensor_tensor(out=hst[:], in0=hst[:], in1=scratch[:],
                                    op=mybir.AluOpType.max)
            nc.sync.dma_start(out=out_d[m * 128:(m + 1) * 128, :], in_=hst[:])

    nc.compile()
    return nc


def _get_program():
    if "nc" not in _CACHE:
        _CACHE["nc"] = _build_program()
    return _CACHE["nc"]


def make_in_maps(pts, W1, g1, b1, W2, g2, b2, W3, g3, b3, W4, g4, b4,
                 W5, g5, b5):
    """Host-side preprocessing: fold BN scale into weights, split the
    EdgeConv concat algebra, shard batch across cores."""
    f = np.float32
    shared = {}
    Ws = [(W1, g1, b1), (W2, g2, b2), (W3, g3, b3), (W4, g4, b4)]
    for li, ((W, g, b), L) in enumerate(zip(Ws, LAYERS), start=1):
        W = np.asarray(W, f) * np.asarray(g, f)[None, :]
        c2 = W.shape[0]
        c = c2 // 2
        wd = W[:c]                # multiplies (x_j - x_i)
        wcn = W[c:] - W[:c]       # multiplies x_i after folding the subtract
        cin = L["cin"]
        wd_p = np.zeros((cin, W.shape[1]), f)
        wc_p = np.zeros((cin, W.shape[1]), f)
        wd_p[:c] = wd
        wc_p[:c] = wcn
        shared[f"wd{li}"] = wd_p
        shared[f"wc{li}"] = wc_p
        b = np.asarray(b, f)
        if b.shape[0] > 128:
            b = b.reshape(-1, 128).T.copy()
        else:
            b = b.reshape(-1, 1)
        shared[f"b{li}"] = np.ascontiguousarray(b)

    W5g = np.asarray(W5, f) * np.asarray(g5, f)[None, :]
    for t, (lo, hi) in zip("abcde", ((0, 64), (64, 128), (128, 256),
                                     (256, 384), (384, 512))):
        shared[f"w5{t}"] = np.ascontiguousarray(W5g[lo:hi])
    shared["b5"] = np.ascontiguousarray(np.asarray(b5, f).reshape(8, 128).T)

    pts = np.asarray(pts, f)
    in_maps = []
    for core in range(NCORES):
        x0 = np.zeros((16, N), f)
        x0[0:3] = pts[core, :, 0:3].T
        in_maps.append({"x0": x0, **shared})
    return in_maps


def kernel(**inputs):
    global LAST_RESULTS
    from concourse.bass_utils import run_bass_kernel_spmd

    nc = _get_program()
    in_maps = make_in_maps(**inputs)
    trace = bool(_CACHE.get("trace"))
    res = run_bass_kernel_spmd(nc, in_maps, list(range(NCORES)), trace=trace)
    LAST_RESULTS = res
    return np.stack([r["out"] for r in res.results]).astype(np.float32)
